# revision 1
# baseline (speedup 1.0000x reference)
"""Trainium2 Bass kernel for the GemNet AtomUpdateBlock (gnn message passing).

Strategy (no collectives needed):
  * Host: stable-sort edges by destination atom id, shard the (padded) atom
    range equally across the 8 cores, and pad each 128-atom block's edge list
    to whole 128-edge tiles (zero messages).  Every core then owns a disjoint
    set of atoms and all edges pointing at them.  The per-edge one-hot
    scatter matrices are precomputed on the host and streamed with the data.
  * Device, per core: for each 128-edge tile
        gate = rbf_tile @ W_rbf          (TensorE bf16, rbf pre-transposed,
                                          row-group packed, pipelined one
                                          super ahead of the consumer)
        x    = m_tile * gate             (VectorE, gate read from PSUM)
        x2  += onehot^T @ x              (TensorE bf16 scatter-matmul into the
                                          owning 128-atom block's PSUM tile)
    Blocks are processed in pairs; each pair's 7-layer MLP (transposed
    layout [feat, atom], bf16 matmuls, f32 PSUM) is chopped into per-layer
    steps that are interleaved between edge supers so the TensorE never
    stalls on the serial layer chain.  All scalar factors (ScaledSiLU 1/0.6,
    residual 1/sqrt2, fitted `scale`) are folded into weights / the final
    scaled f32 copy.  Output is written transposed; host un-transposes.
Precision: bf16 on TensorE paths, f32 accumulation everywhere.
"""

import sys, types, contextlib, ctypes, math
from collections import deque

sys.path.insert(0, "/opt/trn_rl_repo")

import numpy as np
import ml_dtypes

BF16_NP = ml_dtypes.bfloat16


def _install_ntff_hook_shim():
    """bass_utils imports antenv.axon_hooks for trace=True under axon; this
    container's antenv lacks that module.  Recreate the boot hook via ctypes."""
    if "antenv.axon_hooks" in sys.modules:
        return
    try:
        lib = ctypes.CDLL("/opt/axon/libaxon_pjrt.so")
    except OSError:
        lib = None
    hook = None
    if lib is not None and hasattr(lib, "axon_start_nrt_profile"):
        lib.axon_start_nrt_profile.argtypes = [ctypes.POINTER(ctypes.c_int64), ctypes.c_size_t]
        lib.axon_start_nrt_profile.restype = ctypes.c_int64
        lib.axon_stop_nrt_profile.argtypes = [ctypes.c_char_p]
        lib.axon_stop_nrt_profile.restype = ctypes.c_int64

        @contextlib.contextmanager
        def hook(output_dir, device_ids):
            import jax
            jax.devices()
            if device_ids:
                ids = (ctypes.c_int64 * len(device_ids))(*device_ids)
                rc = lib.axon_start_nrt_profile(ids, len(device_ids))
            else:
                rc = lib.axon_start_nrt_profile(None, 0)
            if rc != 0:
                raise RuntimeError(f"axon_start_nrt_profile rc={rc}")
            try:
                yield
            finally:
                n = lib.axon_stop_nrt_profile(str(output_dir).encode())
                print(f"ntff profile: {n} file(s) -> {output_dir}", file=sys.stderr)

    mod = types.ModuleType("antenv.axon_hooks")
    mod.get_axon_ntff_profile_hook = lambda: hook
    mod.set_axon_ntff_profile_hook = lambda h: None
    sys.modules["antenv.axon_hooks"] = mod


_install_ntff_hook_shim()

import concourse.bass as bass
import concourse.tile as tile
import concourse.mybir as mybir
from concourse import bacc
from concourse.alu_op_type import AluOpType
from concourse.bass_utils import run_bass_kernel_spmd

F32 = mybir.dt.float32
BF16 = mybir.dt.bfloat16
FP8 = mybir.dt.float8e4
FP8_NP = ml_dtypes.float8_e4m3fn

N_CORES = 8
N_ATOMS = 50000
N_BLOCKS = 50                  # 128-atom blocks per core (paired for the MLP)
ATOMS_PER_CORE = N_BLOCKS * 128
N_ATOMS_PAD = N_CORES * ATOMS_PER_CORE
N_PAIRS = N_BLOCKS // 2
D = 256                        # feature dim
DR = 16                        # rbf dim
TILE_E = 128                   # edges per tile
SUPER = 2                      # tiles per DVE super-op (one PSUM bank each)
CHUNK = 32                     # tiles per DMA chunk
N_HID = 3

INV_SQRT2 = 0.7071067811865476
S_SILU = 1.0 / 0.6

TRACE = False                  # test.py sets this for profiled runs
ACT_FUNC = "Silu"              # sim_test swaps to Sigmoid (sim lacks Silu)

_cache = {}


# ----------------------------------------------------------------- builder --
def _build(t_list):
    """Build + compile the per-core Bass graph for block tile counts t_list."""
    T = int(sum(t_list))
    assert T % CHUNK == 0
    C = T // CHUNK
    S = T // SUPER
    SUP_PER_CHUNK = CHUNK // SUPER
    # chunk columns (bf16): m | rbfT (row-group packed); onehot ships fp8
    MD_W = CHUNK * D + (CHUNK // 4) * TILE_E
    OH_W = CHUNK * TILE_E

    block_of = np.repeat(np.arange(N_BLOCKS), t_list)
    ends = np.cumsum(t_list)
    starts = ends - np.asarray(t_list)

    nc = bacc.Bacc("TRN2", target_bir_lowering=False, debug=False,
                   num_devices=N_CORES)

    md_d = nc.dram_tensor("md", [C, 128, MD_W], BF16, kind="ExternalInput")
    oh_d = nc.dram_tensor("oh", [C, 128, OH_W], FP8, kind="ExternalInput")
    wmlp_d = nc.dram_tensor("wmlp", [128, 7 * 4 * 128], BF16, kind="ExternalInput")
    wr_d = nc.dram_tensor("wr", [128, D], BF16, kind="ExternalInput")
    ident_d = nc.dram_tensor("ident", [128, 128], BF16, kind="ExternalInput")
    out_d = nc.dram_tensor("out", [N_PAIRS, 2, 128, 2 * 128], F32,
                           kind="ExternalOutput")

    from contextlib import ExitStack

    with tile.TileContext(nc) as tc, ExitStack() as ctx:
        io_pool = ctx.enter_context(tc.tile_pool(name="io", bufs=3))
        ohio_pool = ctx.enter_context(tc.tile_pool(name="ohio", bufs=3))
        x_pool = ctx.enter_context(tc.tile_pool(name="x", bufs=4))
        gsb_pool = ctx.enter_context(tc.tile_pool(name="gsb", bufs=3))
        cst_pool = ctx.enter_context(tc.tile_pool(name="cst", bufs=1))
        x2sb_pool = ctx.enter_context(tc.tile_pool(name="x2sb", bufs=3))
        pairx_pool = ctx.enter_context(tc.tile_pool(name="pairx", bufs=8))
        act_pool = ctx.enter_context(tc.tile_pool(name="acts", bufs=8))
        outt_pool = ctx.enter_context(tc.tile_pool(name="outt", bufs=4))
        gate_pool = ctx.enter_context(tc.tile_pool(name="gate", bufs=2, space="PSUM"))
        x2_pool = ctx.enter_context(tc.tile_pool(name="x2", bufs=2, space="PSUM"))
        mlp_pool = ctx.enter_context(tc.tile_pool(name="mlppsum", bufs=2, space="PSUM"))

        wmlp_sb = cst_pool.tile([128, 7 * 4 * 128], BF16, tag="wmlp")
        nc.sync.dma_start(out=wmlp_sb[:], in_=wmlp_d[:])
        wr_sb = cst_pool.tile([128, D], BF16, tag="wr")
        nc.sync.dma_start(out=wr_sb[:], in_=wr_d[:])
        ident_sb = cst_pool.tile([128, 128], BF16, tag="ident")
        nc.sync.dma_start(out=ident_sb[:], in_=ident_d[:])
        ident_b = cst_pool.tile([128, 128], BF16, tag="identb")
        nc.sync.dma_start(out=ident_b[:], in_=ident_d[:])

        gammas = [1.0, math.sqrt(2.0), 2.0]          # s/alpha_i
        alpha4 = S_SILU * INV_SQRT2 ** 3

        def w_ap(l, kc, oc):
            i = (l * 4 + kc * 2 + oc) * 128
            return wmlp_sb[:, i:i + 128]

        silu_fn = getattr(mybir.ActivationFunctionType, ACT_FUNC)

        # ---------------- MLP pair state machine ----------------
        pair_state = {}    # pair -> dict(X=[2 tiles], cur=[2], Xres=[2])
        steps = deque()    # pending closures, one emitted per super

        def on_block_done(b, x2_ps):
            # x2 PSUM [128 atoms, 256] f32 -> SBUF bf16, transpose into the
            # pair's [128 feat, 256 atoms] input tiles (this block's half).
            p, half = divmod(b, 2)
            if half == 0:
                xp0 = pairx_pool.tile([128, 2 * 128], BF16, tag="pairx")
                xp1 = pairx_pool.tile([128, 2 * 128], BF16, tag="pairx")
                pair_state[p] = {"X": [xp0, xp1]}
            st = pair_state[p]
            x2sb = x2sb_pool.tile([128, D], BF16, tag="x2sb")
            nc.scalar.copy(x2sb[:], x2_ps[:])
            for cidx in range(2):
                tp = mlp_pool.tile([128, 128], BF16, tag="mlppsum")
                nc.tensor.transpose(tp[:], x2sb[:, cidx * 128:(cidx + 1) * 128],
                                    ident_sb[:])
                nc.scalar.copy(st["X"][cidx][:, half * 128:(half + 1) * 128],
                               tp[:])
            if half == 1:
                for l in range(7):
                    steps.append(lambda p=p, l=l: emit_layer(p, l))
                steps.append(lambda p=p: emit_final(p))

        def emit_layer(p, l):
            st = pair_state[p]
            cur = st["cur"] if l > 0 else st["X"]
            new = []
            for oc in range(2):
                z = mlp_pool.tile([128, 2 * 128], F32, tag="mlppsum")
                for kc in range(2):
                    nc.tensor.matmul(z[:], w_ap(l, kc, oc), cur[kc][:],
                                     start=(kc == 0), stop=(kc == 1))
                h = act_pool.tile([128, 2 * 128], BF16, tag="acts")
                nc.scalar.activation(h[:], z[:], silu_fn)
                new.append(h)
            if l == 0:
                st["Xres"] = new
                st["cur"] = new
            elif l % 2 == 1:               # A-layer output
                st["cur"] = new
            else:                          # B-layer output: residual
                i_res = l // 2 - 1
                nxt = []
                for cidx in range(2):
                    xn = act_pool.tile([128, 2 * 128], BF16, tag="acts")
                    nc.vector.scalar_tensor_tensor(
                        xn[:], new[cidx][:], gammas[i_res], st["Xres"][cidx][:],
                        AluOpType.mult, AluOpType.add)
                    nxt.append(xn)
                st["Xres"] = nxt
                st["cur"] = nxt

        def emit_final(p):
            st = pair_state.pop(p)
            for cidx in range(2):
                ot = outt_pool.tile([128, 2 * 128], F32, tag="outt")
                nc.scalar.mul(ot[:], st["Xres"][cidx][:], alpha4)
                nc.gpsimd.dma_start(out=out_d[p, cidx], in_=ot[:])

        # ---------------- edge phase (software pipelined) ----------------
        md_tiles = {}
        oh_tiles = {}

        def emit_dma(c):
            if c in md_tiles or c >= C:
                return
            t_io = io_pool.tile([128, MD_W], BF16, tag="io")
            nc.sync.dma_start(out=t_io[0:64, :], in_=md_d[c][0:64])
            nc.scalar.dma_start(out=t_io[64:128, :], in_=md_d[c][64:128])
            md_tiles[c] = t_io
            t_oh = ohio_pool.tile([128, OH_W], FP8, tag="ohio")
            nc.sync.dma_start(out=t_oh[0:64, :], in_=oh_d[c][0:64])
            nc.scalar.dma_start(out=t_oh[64:128, :], in_=oh_d[c][64:128])
            oh_tiles[c] = t_oh

        gate_tiles = {}

        def emit_gate(s):
            c = s // SUP_PER_CHUNK
            if s % SUP_PER_CHUNK == 0:
                emit_dma(c)
                emit_dma(c + 1)
            md = md_tiles[c]
            gate = gate_pool.tile([128, SUPER * 512], F32, tag="gate")
            for jj in range(SUPER):
                j = (s % SUP_PER_CHUNK) * SUPER + jj
                g = j % 4
                q = j // 4
                rbfT = md[32 * g:32 * g + 32,
                          CHUNK * D + 128 * q:CHUNK * D + 128 * q + 128]
                nc.tensor.matmul(gate[:, jj * 512:jj * 512 + D], rbfT,
                                 wr_sb[32 * g:32 * g + 32, :],
                                 tile_position=(32 * g, 0))
            gate_tiles[s] = gate

        x2_cur = None
        emit_gate(0)
        for s in range(S):
            if s + 1 < S:
                emit_gate(s + 1)
            c = s // SUP_PER_CHUNK
            md = md_tiles[c]
            sp = s % SUP_PER_CHUNK
            gate = gate_tiles.pop(s)
            xsb = x_pool.tile([128, SUPER * D], BF16, tag="x")
            gate_v = gate[:].rearrange("p (a b) -> p a b", b=512)[:, :, 0:D]
            m_v = md[:, sp * SUPER * D:(sp + 1) * SUPER * D] \
                .rearrange("p (a b) -> p a b", b=D)
            x_v = xsb[:].rearrange("p (a b) -> p a b", b=D)
            if s % 3 == 0:
                # offload the PSUM read to ScalarE: gate -> SBUF bf16, then
                # the VectorE multiply runs in 2x mode (all-bf16 SBUF)
                gsb = gsb_pool.tile([128, SUPER * D], BF16, tag="gsb")
                g_sb_v = gsb[:].rearrange("p (a b) -> p a b", b=D)
                nc.scalar.copy(g_sb_v, gate_v)
                nc.vector.tensor_tensor(x_v, m_v, g_sb_v, AluOpType.mult)
            else:
                nc.vector.tensor_tensor(x_v, m_v, gate_v, AluOpType.mult)
            for jj in range(SUPER):
                j = sp * SUPER + jj
                t = c * CHUNK + j
                b = int(block_of[t])
                oh = oh_tiles[c][:, 128 * j:128 * (j + 1)]
                if t == starts[b]:
                    x2_cur = x2_pool.tile([128, D], F32, tag="x2")
                last = (t == ends[b] - 1)
                nc.tensor.matmul(x2_cur[:], oh,
                                 xsb[:, jj * D:(jj + 1) * D],
                                 start=(t == starts[b]), stop=last)
                if last:
                    on_block_done(b, x2_cur)
            if steps:
                steps.popleft()()
        while steps:
            steps.popleft()()

    nc.compile()
    return nc


# ------------------------------------------------------------ host wrapper --
def kernel(h=None, m=None, rbf=None, id_j=None, W_rbf=None, W1=None,
           res_W=None, scale=None, **_unused):
    global LAST_RESULT
    m = np.ascontiguousarray(np.asarray(m, dtype=np.float32))
    rbf = np.ascontiguousarray(np.asarray(rbf, dtype=np.float32))
    ids = np.asarray(id_j).astype(np.int64)
    W_rbf = np.asarray(W_rbf, dtype=np.float32)
    W1 = np.asarray(W1, dtype=np.float32)
    res_W = np.asarray(res_W, dtype=np.float32)
    scale_v = float(np.asarray(scale).reshape(-1)[0])

    nE = ids.shape[0]

    # ---- sort edges by destination atom, shard atoms across cores ----
    perm = np.argsort(ids, kind="stable")
    ids_s = ids[perm]

    core_lo = np.searchsorted(ids_s, np.arange(N_CORES) * ATOMS_PER_CORE)
    core_hi = np.append(core_lo[1:], nE)

    cnts = np.zeros((N_CORES, N_BLOCKS), np.int64)
    per_core = []
    for k in range(N_CORES):
        ids_k = ids_s[core_lo[k]:core_hi[k]] - k * ATOMS_PER_CORE
        blk = ids_k >> 7
        cnts[k] = np.bincount(blk, minlength=N_BLOCKS)
        per_core.append((ids_k, blk))
    t_list = np.maximum(1, np.ceil(cnts.max(axis=0) / TILE_E).astype(np.int64))
    pad = (-int(t_list.sum())) % CHUNK
    t_list[-1] += pad
    T = int(t_list.sum())
    C = T // CHUNK
    MD_W = CHUNK * D + (CHUNK // 4) * TILE_E
    OH_W = CHUNK * TILE_E

    key = tuple(t_list.tolist())
    if key not in _cache:
        _cache[key] = _build(t_list)
    nc = _cache[key]

    offs = np.concatenate([[0], np.cumsum(t_list)[:-1]])   # tile offset per block

    # ---- shared (replicated) small tensors ----
    s, c = S_SILU, INV_SQRT2
    alphas = [s, c * s, c * c * s]
    layersW = [scale_v * W1]
    for i in range(N_HID):
        layersW.append(alphas[i] * res_W[i, 0])
        layersW.append(s * res_W[i, 1])
    wmlp = np.zeros((128, 7 * 4 * 128), np.float32)
    for l in range(7):
        Wl = layersW[l]
        for kc in range(2):
            for oc in range(2):
                i = (l * 4 + kc * 2 + oc) * 128
                wmlp[:, i:i + 128] = Wl[kc * 128:(kc + 1) * 128,
                                        oc * 128:(oc + 1) * 128]
    wmlp = wmlp.astype(BF16_NP)
    wr_rep = np.zeros((128, D), np.float32)
    for g in range(4):
        wr_rep[32 * g:32 * g + DR, :] = W_rbf
    wr_rep = wr_rep.astype(BF16_NP)
    ident = np.eye(128, dtype=BF16_NP)

    # ---- per-core big tensors ----
    in_maps = []
    for k in range(N_CORES):
        ids_k, blk = per_core[k]
        nk = len(ids_k)
        rank = np.arange(nk) - np.searchsorted(blk, blk)
        pos = offs[blk] * TILE_E + rank

        m_pad = np.zeros((T * TILE_E, D), BF16_NP)
        rbf_pad = np.zeros((T * TILE_E, DR), BF16_NP)
        idr_pad = np.zeros(T * TILE_E, np.int32)
        sel = perm[core_lo[k]:core_hi[k]]
        m_pad[pos] = m[sel].astype(BF16_NP)
        rbf_pad[pos] = rbf[sel].astype(BF16_NP)
        idr_pad[pos] = ids_k - (blk << 7)

        md = np.empty((C, 128, MD_W), BF16_NP)
        md[:, :, :CHUNK * D] = m_pad.reshape(C, CHUNK, TILE_E, D) \
            .transpose(0, 2, 1, 3).reshape(C, 128, CHUNK * D)
        a = rbf_pad.reshape(C, CHUNK // 4, 4, TILE_E, DR).transpose(0, 2, 4, 1, 3)
        md_r = np.zeros((C, 4, 32, CHUNK // 4, TILE_E), BF16_NP)
        md_r[:, :, :DR] = a
        md[:, :, CHUNK * D:] = md_r.reshape(C, 128, (CHUNK // 4) * TILE_E)
        onehot = (idr_pad.reshape(T, TILE_E, 1)
                  == np.arange(TILE_E, dtype=np.int32)).astype(FP8_NP)
        oh = np.ascontiguousarray(
            onehot.reshape(C, CHUNK, TILE_E, TILE_E)
            .transpose(0, 2, 1, 3).reshape(C, 128, CHUNK * TILE_E))

        in_maps.append({
            "md": md, "oh": oh, "wmlp": wmlp, "wr": wr_rep, "ident": ident,
        })

    res = run_bass_kernel_spmd(nc, in_maps, list(range(N_CORES)), trace=TRACE)
    LAST_RESULT = res

    # ---- reassemble ----
    out = np.empty((N_ATOMS_PAD, D), np.float32)
    for k in range(N_CORES):
        od = res.results[k]["out"]                     # [25, 2, 128, 256]
        out[k * ATOMS_PER_CORE:(k + 1) * ATOMS_PER_CORE] = \
            od.transpose(0, 3, 1, 2).reshape(ATOMS_PER_CORE, D)
    return out[:N_ATOMS]


LAST_RESULT = None



# revision 3
# speedup vs baseline: 1.2812x; 1.2812x over previous
"""Trainium2 Bass kernel for the GemNet AtomUpdateBlock (gnn message passing).

v2 strategy (no collectives):
  * Host: stable-sort edges by destination atom, shard the (padded) atom
    range across 8 cores.  Within each 128-atom block, each atom's edge
    list is padded to EVEN length and consecutive edge pairs share one
    "slot": a pair-tile holds 128 slots x 2 edges x 256 feats.  One fp8
    one-hot [slot -> atom] stationary then serves BOTH edges of every
    slot (half the LDWEIGHTS + half the one-hot DMA of v1).
  * Gate: rbf^T for 4 pair-tiles (8 edge sets) is packed into one
    [128,128] stationary (4 row-groups x {even 16 rows, odd 16 rows});
    a block-diagonal [[W,0],[0,W]] moving operand computes a pair-tile's
    full [128 slot, 512] gate in ONE N=512 matmul (rotating row-groups
    let LDWEIGHTS pull ahead of in-flight matmuls).
  * x = m .* gate on DVE; alternate supers route the PSUM->SBUF gate
    copy through ScalarE so the DVE multiply runs in 2x bf16 mode.
  * Scatter: per pair-tile 2 accumulating matmuls (even/odd feature
    halves) into a per-block [128 atom, 256] PSUM tile.
  * MLP over groups of 4 blocks (512 atoms, N=512 matmuls), 7 layers,
    interleaved between edge supers.  Scalar factors folded into
    weights / final scaled copy.  Output written transposed; host
    un-transposes.
Precision: bf16 on TensorE paths, f32 accumulation everywhere.
"""

import sys, types, contextlib, ctypes, math
from collections import deque

sys.path.insert(0, "/opt/trn_rl_repo")

import numpy as np
import ml_dtypes

BF16_NP = ml_dtypes.bfloat16


def _install_ntff_hook_shim():
    """bass_utils imports antenv.axon_hooks for trace=True under axon; this
    container's antenv lacks that module.  Recreate the boot hook via ctypes."""
    if "antenv.axon_hooks" in sys.modules:
        return
    try:
        lib = ctypes.CDLL("/opt/axon/libaxon_pjrt.so")
    except OSError:
        lib = None
    hook = None
    if lib is not None and hasattr(lib, "axon_start_nrt_profile"):
        lib.axon_start_nrt_profile.argtypes = [ctypes.POINTER(ctypes.c_int64), ctypes.c_size_t]
        lib.axon_start_nrt_profile.restype = ctypes.c_int64
        lib.axon_stop_nrt_profile.argtypes = [ctypes.c_char_p]
        lib.axon_stop_nrt_profile.restype = ctypes.c_int64

        @contextlib.contextmanager
        def hook(output_dir, device_ids):
            import jax
            jax.devices()
            if device_ids:
                ids = (ctypes.c_int64 * len(device_ids))(*device_ids)
                rc = lib.axon_start_nrt_profile(ids, len(device_ids))
            else:
                rc = lib.axon_start_nrt_profile(None, 0)
            if rc != 0:
                raise RuntimeError(f"axon_start_nrt_profile rc={rc}")
            try:
                yield
            finally:
                n = lib.axon_stop_nrt_profile(str(output_dir).encode())
                print(f"ntff profile: {n} file(s) -> {output_dir}", file=sys.stderr)

    mod = types.ModuleType("antenv.axon_hooks")
    mod.get_axon_ntff_profile_hook = lambda: hook
    mod.set_axon_ntff_profile_hook = lambda h: None
    sys.modules["antenv.axon_hooks"] = mod


_install_ntff_hook_shim()

import concourse.bass as bass
import concourse.tile as tile
import concourse.mybir as mybir
from concourse import bacc
from concourse.alu_op_type import AluOpType
from concourse.bass_utils import run_bass_kernel_spmd

F32 = mybir.dt.float32
BF16 = mybir.dt.bfloat16
FP8 = mybir.dt.float8e4
FP8_NP = ml_dtypes.float8_e4m3fn

N_CORES = 8
N_ATOMS = 50000
N_BLOCKS = 50                  # 128-atom blocks per core
ATOMS_PER_CORE = N_BLOCKS * 128
N_ATOMS_PAD = N_CORES * ATOMS_PER_CORE
D = 256                        # feature dim
DR = 16                        # rbf dim
TILE_S = 128                   # slots per pair-tile (2 edges per slot)
SUPER = 2                      # pair-tiles per super
CHUNK = 32                     # pair-tiles per DMA chunk
N_HID = 3

# MLP groups of 4 blocks (512 atoms); 50 blocks -> 12 quads + 1 pair
GROUPS = [list(range(4 * g, 4 * g + 4)) for g in range(12)] + [[48, 49]]
NG = len(GROUPS)
GROUP_OF = {}
for _gi, _bs in enumerate(GROUPS):
    for _h, _b in enumerate(_bs):
        GROUP_OF[_b] = (_gi, _h)

INV_SQRT2 = 0.7071067811865476
S_SILU = 1.0 / 0.6

TRACE = False                  # test.py sets this for profiled runs
ACT_FUNC = "Silu"              # sim_test swaps to Sigmoid (sim lacks Silu)

_cache = {}


# ----------------------------------------------------------------- builder --
def _build(t_list):
    """Build + compile the per-core Bass graph for block pair-tile counts."""
    T = int(sum(t_list))
    assert T % CHUNK == 0
    C = T // CHUNK
    S = T // SUPER
    SUP_PER_CHUNK = CHUNK // SUPER
    # chunk columns (bf16): m (CHUNK pair-tiles x [2,256]) | rbf quad packs
    M_W = CHUNK * 2 * D
    RB_W = (CHUNK // 4) * TILE_S
    MD_W = M_W + RB_W
    OH_W = CHUNK * TILE_S

    block_of = np.repeat(np.arange(N_BLOCKS), t_list)
    ends = np.cumsum(t_list)
    starts = ends - np.asarray(t_list)

    nc = bacc.Bacc("TRN2", target_bir_lowering=False, debug=False,
                   num_devices=N_CORES)

    md_d = nc.dram_tensor("md", [C, 128, MD_W], BF16, kind="ExternalInput")
    oh_d = nc.dram_tensor("oh", [C, 128, OH_W], FP8, kind="ExternalInput")
    wmlp_d = nc.dram_tensor("wmlp", [128, 7 * 4 * 128], BF16, kind="ExternalInput")
    wrb_d = nc.dram_tensor("wrb", [128, 2 * D], BF16, kind="ExternalInput")
    ident_d = nc.dram_tensor("ident", [128, 128], BF16, kind="ExternalInput")
    out_d = nc.dram_tensor("out", [NG, 2, 128, 512], F32,
                           kind="ExternalOutput")

    from contextlib import ExitStack

    with tile.TileContext(nc) as tc, ExitStack() as ctx:
        io_pool = ctx.enter_context(tc.tile_pool(name="io", bufs=3))
        ohio_pool = ctx.enter_context(tc.tile_pool(name="ohio", bufs=3))
        x_pool = ctx.enter_context(tc.tile_pool(name="x", bufs=4))
        gsb_pool = ctx.enter_context(tc.tile_pool(name="gsb", bufs=3))
        cst_pool = ctx.enter_context(tc.tile_pool(name="cst", bufs=1))
        x2sb_pool = ctx.enter_context(tc.tile_pool(name="x2sb", bufs=3))
        pairx_pool = ctx.enter_context(tc.tile_pool(name="pairx", bufs=6))
        act_pool = ctx.enter_context(tc.tile_pool(name="acts", bufs=8))
        outt_pool = ctx.enter_context(tc.tile_pool(name="outt", bufs=3))
        gate_pool = ctx.enter_context(tc.tile_pool(name="gate", bufs=2, space="PSUM"))
        x2_pool = ctx.enter_context(tc.tile_pool(name="x2", bufs=2, space="PSUM"))
        mlp_pool = ctx.enter_context(tc.tile_pool(name="mlppsum", bufs=2, space="PSUM"))

        wmlp_sb = cst_pool.tile([128, 7 * 4 * 128], BF16, tag="wmlp")
        nc.sync.dma_start(out=wmlp_sb[:], in_=wmlp_d[:])
        wrb_sb = cst_pool.tile([128, 2 * D], BF16, tag="wrb")
        nc.sync.dma_start(out=wrb_sb[:], in_=wrb_d[:])
        ident_sb = cst_pool.tile([128, 128], BF16, tag="ident")
        nc.sync.dma_start(out=ident_sb[:], in_=ident_d[:])

        gammas = [1.0, math.sqrt(2.0), 2.0]          # s/alpha_i
        alpha4 = S_SILU * INV_SQRT2 ** 3

        def w_ap(l, kc, oc):
            i = (l * 4 + kc * 2 + oc) * 128
            return wmlp_sb[:, i:i + 128]

        silu_fn = getattr(mybir.ActivationFunctionType, ACT_FUNC)

        # ---------------- MLP group state machine ----------------
        grp_state = {}     # gi -> dict(X=[2 tiles], cur, Xres)
        steps = deque()    # pending closures, one emitted per super

        def on_block_done(b, x2_ps):
            # x2 PSUM [128 atoms, 256] f32 -> SBUF bf16 -> transpose into the
            # group's [128 feat, 512 atoms] input tiles (this block's column).
            gi, h = GROUP_OF[b]
            w = 128 * len(GROUPS[gi])
            if h == 0:
                xp0 = pairx_pool.tile([128, 512], BF16, tag="pairx")
                xp1 = pairx_pool.tile([128, 512], BF16, tag="pairx")
                grp_state[gi] = {"X": [xp0, xp1], "w": w}
            st = grp_state[gi]
            x2sb = x2sb_pool.tile([128, D], BF16, tag="x2sb")
            nc.vector.tensor_copy(x2sb[:], x2_ps[:])
            for cidx in range(2):
                tp = mlp_pool.tile([128, 128], BF16, tag="mlppsum")
                nc.tensor.transpose(tp[:], x2sb[:, cidx * 128:(cidx + 1) * 128],
                                    ident_sb[:])
                if cidx == 0:
                    nc.vector.tensor_copy(
                        st["X"][cidx][:, h * 128:(h + 1) * 128], tp[:])
                else:
                    nc.scalar.copy(
                        st["X"][cidx][:, h * 128:(h + 1) * 128], tp[:])
            if h == len(GROUPS[gi]) - 1:
                for l in range(7):
                    steps.append(lambda gi=gi, l=l: emit_layer(gi, l))
                steps.append(lambda gi=gi: emit_final(gi))

        def emit_layer(gi, l):
            st = grp_state[gi]
            w = st["w"]
            cur = st["cur"] if l > 0 else st["X"]
            new = []
            for oc in range(2):
                z = mlp_pool.tile([128, 512], F32, tag="mlppsum")
                for kc in range(2):
                    nc.tensor.matmul(z[:, :w], w_ap(l, kc, oc), cur[kc][:, :w],
                                     start=(kc == 0), stop=(kc == 1))
                h = act_pool.tile([128, 512], BF16, tag="acts")
                nc.scalar.activation(h[:, :w], z[:, :w], silu_fn)
                new.append(h)
            if l == 0:
                st["Xres"] = new
                st["cur"] = new
            elif l % 2 == 1:               # A-layer output
                st["cur"] = new
            else:                          # B-layer output: residual
                i_res = l // 2 - 1
                nxt = []
                for cidx in range(2):
                    xn = act_pool.tile([128, 512], BF16, tag="acts")
                    nc.vector.scalar_tensor_tensor(
                        xn[:, :w], new[cidx][:, :w], gammas[i_res],
                        st["Xres"][cidx][:, :w],
                        AluOpType.mult, AluOpType.add)
                    nxt.append(xn)
                st["Xres"] = nxt
                st["cur"] = nxt

        def emit_final(gi):
            st = grp_state.pop(gi)
            w = st["w"]
            for cidx in range(2):
                ot = outt_pool.tile([128, 512], F32, tag="outt")
                nc.scalar.mul(ot[:, :w], st["Xres"][cidx][:, :w], alpha4)
                nc.gpsimd.dma_start(out=out_d[gi, cidx, :, 0:w], in_=ot[:, :w])

        # ---------------- edge phase (software pipelined) ----------------
        md_tiles = {}
        oh_tiles = {}

        def emit_dma(c):
            if c in md_tiles or c >= C:
                return
            t_io = io_pool.tile([128, MD_W], BF16, tag="io")
            nc.sync.dma_start(out=t_io[:], in_=md_d[c])
            md_tiles[c] = t_io
            t_oh = ohio_pool.tile([128, OH_W], FP8, tag="ohio")
            nc.scalar.dma_start(out=t_oh[:], in_=oh_d[c])
            oh_tiles[c] = t_oh

        gate_tiles = {}

        def emit_gate(s):
            c = s // SUP_PER_CHUNK
            if s % SUP_PER_CHUNK == 0:
                emit_dma(c)
                emit_dma(c + 1)
            md = md_tiles[c]
            gate = gate_pool.tile([128, SUPER * 2 * D], F32, tag="gate")
            for jj in range(SUPER):
                t = s * SUPER + jj
                g = t % 4
                q = (t % CHUNK) // 4
                rbf_stat = md[32 * g:32 * g + 32,
                              M_W + q * TILE_S:M_W + (q + 1) * TILE_S]
                nc.tensor.matmul(gate[:, jj * 512:(jj + 1) * 512], rbf_stat,
                                 wrb_sb[32 * g:32 * g + 32, :],
                                 tile_position=(32 * g, 0))
            gate_tiles[s] = gate

        x2_cur = None
        emit_gate(0)
        for s in range(S):
            if s + 1 < S:
                emit_gate(s + 1)
            c = s // SUP_PER_CHUNK
            md = md_tiles[c]
            sp = s % SUP_PER_CHUNK
            gate = gate_tiles.pop(s)
            xsb = x_pool.tile([128, SUPER * 2 * D], BF16, tag="x")
            m_v = md[:, sp * SUPER * 2 * D:(sp + 1) * SUPER * 2 * D]
            if s % 2 == 0:
                # offload the PSUM read to ScalarE: gate -> SBUF bf16, then
                # the VectorE multiply runs in 2x mode (all-bf16 SBUF)
                gsb = gsb_pool.tile([128, SUPER * 2 * D], BF16, tag="gsb")
                nc.scalar.copy(gsb[:], gate[:])
                nc.vector.tensor_tensor(xsb[:], m_v, gsb[:], AluOpType.mult)
            else:
                nc.vector.tensor_tensor(xsb[:], m_v, gate[:], AluOpType.mult)
            for jj in range(SUPER):
                t = s * SUPER + jj
                b = int(block_of[t])
                oh = oh_tiles[c][:, TILE_S * (t % CHUNK):TILE_S * (t % CHUNK + 1)]
                if t == starts[b]:
                    x2_cur = x2_pool.tile([128, D], F32, tag="x2")
                last = (t == ends[b] - 1)
                nc.tensor.matmul(x2_cur[:], oh,
                                 xsb[:, jj * 512:jj * 512 + D],
                                 start=(t == starts[b]), stop=False)
                nc.tensor.matmul(x2_cur[:], oh,
                                 xsb[:, jj * 512 + D:(jj + 1) * 512],
                                 start=False, stop=last)
                if last:
                    on_block_done(b, x2_cur)
            if steps:
                steps.popleft()()
        while steps:
            steps.popleft()()

    nc.compile()
    return nc


# ------------------------------------------------------------ host wrapper --
def kernel(h=None, m=None, rbf=None, id_j=None, W_rbf=None, W1=None,
           res_W=None, scale=None, **_unused):
    global LAST_RESULT
    m = np.ascontiguousarray(np.asarray(m, dtype=np.float32))
    rbf = np.ascontiguousarray(np.asarray(rbf, dtype=np.float32))
    ids = np.asarray(id_j).astype(np.int64)
    W_rbf = np.asarray(W_rbf, dtype=np.float32)
    W1 = np.asarray(W1, dtype=np.float32)
    res_W = np.asarray(res_W, dtype=np.float32)
    scale_v = float(np.asarray(scale).reshape(-1)[0])

    nE = ids.shape[0]

    # ---- sort edges by destination atom, shard atoms across cores ----
    perm = np.argsort(ids, kind="stable")
    ids_s = ids[perm]

    core_lo = np.searchsorted(ids_s, np.arange(N_CORES) * ATOMS_PER_CORE)
    core_hi = np.append(core_lo[1:], nE)

    # per-core pair-slot assignment
    per_core = []
    cnts_pairs = np.zeros((N_CORES, N_BLOCKS), np.int64)
    for k in range(N_CORES):
        ids_k = ids_s[core_lo[k]:core_hi[k]] - k * ATOMS_PER_CORE
        nk = len(ids_k)
        # within-atom rank
        first = np.searchsorted(ids_k, ids_k)        # index of first occ
        r = np.arange(nk) - first
        p_atom = r >> 1
        half = (r & 1).astype(np.int64)
        # slots per atom / per block
        cnt = np.bincount(ids_k, minlength=ATOMS_PER_CORE)
        s_cnt = (cnt + 1) >> 1
        off_atom = np.cumsum(s_cnt) - s_cnt          # global slot offset
        blk = ids_k >> 7
        blk_first_atom = (blk << 7)
        blk_slot_start_per_atom = off_atom[blk_first_atom]
        slot_in_block = off_atom[ids_k] - blk_slot_start_per_atom + p_atom
        sc = np.add.reduceat(s_cnt, np.arange(0, ATOMS_PER_CORE, 128))
        cnts_pairs[k] = sc
        per_core.append((ids_k, blk, slot_in_block, half))

    t_list = np.maximum(1, np.ceil(cnts_pairs.max(axis=0) / TILE_S).astype(np.int64))
    pad = (-int(t_list.sum())) % CHUNK
    t_list[-1] += pad
    T = int(t_list.sum())
    C = T // CHUNK
    M_W = CHUNK * 2 * D
    RB_W = (CHUNK // 4) * TILE_S
    MD_W = M_W + RB_W
    OH_W = CHUNK * TILE_S

    key = tuple(t_list.tolist())
    if key not in _cache:
        _cache[key] = _build(t_list)
    nc = _cache[key]

    offs = np.concatenate([[0], np.cumsum(t_list)[:-1]])   # tile offset per block

    # ---- shared (replicated) small tensors ----
    s_, c_ = S_SILU, INV_SQRT2
    alphas = [s_, c_ * s_, c_ * c_ * s_]
    layersW = [scale_v * W1]
    for i in range(N_HID):
        layersW.append(alphas[i] * res_W[i, 0])
        layersW.append(s_ * res_W[i, 1])
    wmlp = np.zeros((128, 7 * 4 * 128), np.float32)
    for l in range(7):
        Wl = layersW[l]
        for kc in range(2):
            for oc in range(2):
                i = (l * 4 + kc * 2 + oc) * 128
                wmlp[:, i:i + 128] = Wl[kc * 128:(kc + 1) * 128,
                                        oc * 128:(oc + 1) * 128]
    wmlp = wmlp.astype(BF16_NP)
    # block-diagonal [[W,0],[0,W]] moving operand, replicated per row-group
    wrb = np.zeros((128, 2 * D), np.float32)
    for g in range(4):
        wrb[32 * g:32 * g + DR, 0:D] = W_rbf
        wrb[32 * g + 16:32 * g + 16 + DR, D:2 * D] = W_rbf
    wrb = wrb.astype(BF16_NP)
    ident = np.eye(128, dtype=BF16_NP)

    # ---- per-core big tensors ----
    m_bf = None
    in_maps = []
    for k in range(N_CORES):
        ids_k, blk, slot_in_block, half = per_core[k]
        sel = perm[core_lo[k]:core_hi[k]]
        tile_i = offs[blk] + (slot_in_block >> 7)
        part_i = slot_in_block & 127

        m_arr = np.zeros((T, 128, 2, D), BF16_NP)
        m_arr[tile_i, part_i, half] = m[sel].astype(BF16_NP)
        rbfT = np.zeros((T // 4, 4, 2, 16, TILE_S), BF16_NP)
        rbfT[tile_i >> 2, tile_i & 3, half, :, part_i] = rbf[sel].astype(BF16_NP)
        onehot = np.zeros((T, 128, TILE_S), FP8_NP)
        e0 = half == 0
        onehot[tile_i[e0], part_i[e0], ids_k[e0] & 127] = 1.0

        md = np.empty((C, 128, MD_W), BF16_NP)
        md[:, :, :M_W] = m_arr.reshape(C, CHUNK, 128, 2 * D) \
            .transpose(0, 2, 1, 3).reshape(C, 128, M_W)
        md[:, :, M_W:] = rbfT.reshape(C, CHUNK // 4, 128, TILE_S) \
            .transpose(0, 2, 1, 3).reshape(C, 128, RB_W)
        oh = np.ascontiguousarray(
            onehot.reshape(C, CHUNK, 128, TILE_S)
            .transpose(0, 2, 1, 3).reshape(C, 128, OH_W))

        in_maps.append({
            "md": md, "oh": oh, "wmlp": wmlp, "wrb": wrb, "ident": ident,
        })

    res = run_bass_kernel_spmd(nc, in_maps, list(range(N_CORES)), trace=TRACE)
    LAST_RESULT = res

    # ---- reassemble ----
    out = np.empty((N_ATOMS_PAD, D), np.float32)
    for k in range(N_CORES):
        od = res.results[k]["out"]                     # [NG, 2, 128, 512]
        for gi, bs in enumerate(GROUPS):
            w = 128 * len(bs)
            arr = od[gi, :, :, :w]                     # [2, 128, w]
            arr = arr.reshape(2, 128, len(bs), 128).transpose(2, 3, 0, 1) \
                .reshape(len(bs) * 128, D)
            a0 = k * ATOMS_PER_CORE + bs[0] * 128
            out[a0:a0 + len(bs) * 128] = arr
    return out[:N_ATOMS]


LAST_RESULT = None
